# revision 14
# baseline (speedup 1.0000x reference)
"""AttentionBlock (GroupNorm + single-head full attention + residual) on 8 trn2 cores.

Sharding: core i -> batch i//4, query strip (i%4)*1024 .. +1024.
Each core computes its batch's full K/V (duplicated across the 4 cores sharing
the batch) so no inter-core communication is needed.

All matmuls run as float32r (full-rate fp32 variant); softmax in fp32.
Softmax skips max-subtraction (scores are O(+-10) with normalized inputs, and
softmax is shift-invariant so the result matches the reference), k-bias is
dropped (shift-invariant), v/proj biases are folded into a rank-1 post-
projection bias, and the softmax row normalization is deferred to the
projection output (row scaling commutes through out @ wp).
"""

import numpy as np
from contextlib import ExitStack

import concourse.bass as bass
import concourse.bacc as bacc
import concourse.tile as tile
from concourse import mybir
from concourse.bass_utils import run_bass_kernel_spmd

B, H, W, C = 2, 64, 64, 512
T = H * W                 # 4096 tokens per batch
NCORES = 8
QS = 1024                 # queries per core
GROUPS, GSIZE = 32, 16    # 8 groups per 128-channel chunk
EPS = 1e-5
SCALE = float(C) ** -0.5
F32 = mybir.dt.float32
F32R = mybir.dt.float32r
import os
if os.environ.get('KERNEL_MM_F32'):
    F32R = mybir.dt.float32
NCH = C // 128            # 4 channel chunks
NW = T // 512             # 8 token windows per batch
NQW = QS // 512           # 2 query windows per core
NBLK = QS // 256          # 4 attention q-blocks of 256 queries


def _r(ap):
    return ap.bitcast(F32R)


def _build():
    nc = bacc.Bacc(None, target_bir_lowering=False)

    xkv_h = nc.declare_dram_parameter("xkv", [T, C], F32, isOutput=False)
    xq_h = nc.declare_dram_parameter("xq", [QS, C], F32, isOutput=False)
    wq_h = nc.declare_dram_parameter("wq", [C, C], F32R, isOutput=False)
    wk_h = nc.declare_dram_parameter("wk", [C, C], F32R, isOutput=False)
    wv_h = nc.declare_dram_parameter("wv", [C, C], F32R, isOutput=False)
    wp_h = nc.declare_dram_parameter("wp", [C, C], F32R, isOutput=False)
    bq_h = nc.declare_dram_parameter("bq", [C], F32, isOutput=False)
    bv_h = nc.declare_dram_parameter("bv", [C], F32, isOutput=False)
    bp_h = nc.declare_dram_parameter("bp", [C], F32, isOutput=False)
    gamma_h = nc.declare_dram_parameter("gamma", [C], F32, isOutput=False)
    beta_h = nc.declare_dram_parameter("beta", [C], F32, isOutput=False)
    ident_h = nc.declare_dram_parameter("ident", [128, 128], F32, isOutput=False)
    g_h = nc.declare_dram_parameter("gmat", [128, 8], F32, isOutput=False)
    gt_h = nc.declare_dram_parameter("gtmat", [8, 128], F32, isOutput=False)
    out_h = nc.declare_dram_parameter("out", [QS, C], F32, isOutput=True)
    DEBUG = bool(os.environ.get("KERNEL_DEBUG"))
    if DEBUG:
        dbg_h = nc.declare_dram_parameter("dbg_h", [128, 512], F32, isOutput=True)
        dbg_kt = nc.declare_dram_parameter("dbg_kt", [128, 512], F32, isOutput=True)
        dbg_q = nc.declare_dram_parameter("dbg_q", [128, 512], F32, isOutput=True)
        dbg_p = nc.declare_dram_parameter("dbg_p", [128, 512], F32, isOutput=True)
        dbg_ot = nc.declare_dram_parameter("dbg_ot", [128, 4, 256], F32, isOutput=True)
        dbg_r = nc.declare_dram_parameter("dbg_r", [128, 1], F32, isOutput=True)
        dbg_sc = nc.declare_dram_parameter("dbg_sc", [128, 2], F32, isOutput=True)

    with tile.TileContext(nc) as tc, ExitStack() as ctx:
        persist = ctx.enter_context(tc.tile_pool(name="persist", bufs=1))
        small = ctx.enter_context(tc.tile_pool(name="small", bufs=1))

        # ---- constants / weights ----
        ident = persist.tile([128, 128], F32, tag="ident", name="ident")
        nc.sync.dma_start(out=ident, in_=ident_h[:, :])
        g_sb = persist.tile([128, 8], F32, tag="gmat", name="gmat")
        nc.sync.dma_start(out=g_sb, in_=g_h[:, :])
        gt_sb = persist.tile([8, 128], F32, tag="gtmat", name="gtmat")
        nc.sync.dma_start(out=gt_sb, in_=gt_h[:, :])

        bigpool = ctx.enter_context(tc.tile_pool(name="bigpool", bufs=1))
        kt_t = [bigpool.tile([128, T], F32R, tag=f"kt{j}", name=f"kt{j}") for j in range(NCH)]
        qts_t = [bigpool.tile([128, QS], F32R, tag=f"qts{j}", name=f"qts{j}") for j in range(NCH)]
        v_big = bigpool.tile([128, T // 128, C], F32R, tag="vbig", name="vbig")
        ctx2 = ExitStack()
        wpool = ctx2.enter_context(tc.tile_pool(name="wpool", bufs=1))
        wq_t = [wpool.tile([128, C], F32R, tag=f"wq{j}", name=f"wq{j}") for j in range(NCH)]
        wk_t = [wpool.tile([128, C], F32R, tag=f"wk{j}", name=f"wk{j}") for j in range(NCH)]
        wv_t = [wpool.tile([128, C], F32R, tag=f"wv{j}", name=f"wv{j}") for j in range(NCH)]
        wp_t = [persist.tile([128, C], F32R, tag=f"wp{j}", name=f"wp{j}") for j in range(NCH)]
        for j in range(NCH):
            sl = slice(j * 128, (j + 1) * 128)
            nc.sync.dma_start(out=wq_t[j], in_=wq_h[sl, :])
            nc.sync.dma_start(out=wk_t[j], in_=wk_h[sl, :])
            nc.sync.dma_start(out=wv_t[j], in_=wv_h[sl, :])
            nc.sync.dma_start(out=wp_t[j], in_=wp_h[sl, :])

        # per-channel vectors as [128, NCH] (column j = channel chunk j)
        def vec_tile(h, name):
            t = small.tile([128, NCH], F32, tag=name)
            nc.sync.dma_start(out=t, in_=h.rearrange("(a p) -> p a", p=128))
            return t

        gamma_sb = vec_tile(gamma_h, "gamma")
        beta_sb = vec_tile(beta_h, "beta")
        bq_sb = vec_tile(bq_h, "bq")
        bv_sb = vec_tile(bv_h, "bv")
        bp_row = small.tile([1, C], F32, tag="bprow", name="bprow")
        nc.sync.dma_start(out=bp_row, in_=bp_h.rearrange("(a c) -> a c", a=1))

        sbq = small.tile([128, NCH], F32, tag="sbq", name="sbq")
        nc.vector.tensor_scalar_mul(sbq, bq_sb, SCALE)
        eps8 = small.tile([8, 1], F32, tag="eps8", name="eps8")
        nc.vector.memset(eps8, EPS)

        rinv_t = [small.tile([128, 1], F32, tag=f"rinv{s}", name=f"rinv{s}") for s in range(2 * NBLK)]

        scale_t = [small.tile([128, 1], F32, tag=f"gnsc{j}", name=f"gnsc{j}") for j in range(NCH)]
        bias_t = [small.tile([128, 1], F32, tag=f"gnbi{j}", name=f"gnbi{j}") for j in range(NCH)]

        # ================= P1: group-norm statistics =================
        with tc.tile_pool(name="p1ps", bufs=1, space="PSUM") as p1ps, \
             tc.tile_pool(name="p1sb", bufs=2) as p1sb, \
             tc.tile_pool(name="statsb", bufs=1) as statsb:
            stats_t = [statsb.tile([128, NW, 6], F32, tag=f"st{j}", name=f"st{j}") for j in range(NCH)]
            for w in range(NW):
                xt4 = []
                for i in range(4):
                    xt = p1sb.tile([128, C], F32, tag="xtok", name="xtok", bufs=5)
                    r0 = w * 512 + i * 128
                    nc.sync.dma_start(out=xt, in_=xkv_h[r0:r0 + 128, :])
                    xt4.append(xt)
                for j in range(NCH):
                    psx = p1ps.tile([128, 512], F32, tag="tp", name="tp", bufs=4)
                    for i in range(4):
                        nc.tensor.transpose(
                            psx[:, i * 128:(i + 1) * 128],
                            xt4[i][:, j * 128:(j + 1) * 128], ident)
                    nc.vector.bn_stats(out=stats_t[j][:, w, :], in_=psx)

            for j in range(NCH):
                mv = p1sb.tile([128, 2], F32, tag="mv", name="mv")
                nc.vector.bn_aggr(out=mv, in_=stats_t[j])
                msq = p1sb.tile([128, 1], F32, tag="msq", name="msq")
                nc.vector.tensor_tensor(out=msq, in0=mv[:, 0:1], in1=mv[:, 0:1],
                                        op=mybir.AluOpType.mult)
                smm = p1sb.tile([128, 2], F32, tag="smm", name="smm")
                nc.vector.tensor_copy(smm[:, 0:1], mv[:, 0:1])
                nc.vector.tensor_tensor(out=smm[:, 1:2], in0=mv[:, 1:2], in1=msq,
                                        op=mybir.AluOpType.add)
                gps = p1ps.tile([8, 2], F32, tag="grp", name="grp", bufs=2)
                nc.tensor.matmul(gps, g_sb, smm, start=True, stop=True)
                gm = p1sb.tile([8, 2], F32, tag="gm", name="gm")
                nc.scalar.copy(gm, gps)
                gmsq = p1sb.tile([8, 1], F32, tag="gmsq", name="gmsq")
                nc.vector.tensor_tensor(out=gmsq, in0=gm[:, 0:1], in1=gm[:, 0:1],
                                        op=mybir.AluOpType.mult)
                vg = p1sb.tile([8, 1], F32, tag="vg", name="vg")
                nc.vector.tensor_tensor(out=vg, in0=gm[:, 1:2], in1=gmsq,
                                        op=mybir.AluOpType.subtract)
                ve = p1sb.tile([8, 1], F32, tag="ve", name="ve")
                nc.vector.tensor_scalar_add(ve, vg, EPS)
                sd = p1sb.tile([8, 1], F32, tag="sd", name="sd")
                nc.scalar.activation(sd, ve, mybir.ActivationFunctionType.Sqrt)
                y0 = p1sb.tile([8, 1], F32, tag="y0", name="y0")
                nc.vector.reciprocal(y0, sd)
                # Newton-Raphson: y = y0 * (1.5 - 0.5 * v * y0^2)
                t1 = p1sb.tile([8, 1], F32, tag="t1", name="t1")
                nc.vector.tensor_tensor(out=t1, in0=ve, in1=y0,
                                        op=mybir.AluOpType.mult)
                nc.vector.tensor_tensor(out=t1, in0=t1, in1=y0,
                                        op=mybir.AluOpType.mult)
                nc.vector.tensor_scalar(out=t1, in0=t1, scalar1=-0.5, scalar2=1.5,
                                        op0=mybir.AluOpType.mult,
                                        op1=mybir.AluOpType.add)
                rstd = p1sb.tile([8, 1], F32, tag="rstd", name="rstd")
                nc.vector.tensor_tensor(out=rstd, in0=y0, in1=t1,
                                        op=mybir.AluOpType.mult)
                gmr = p1sb.tile([8, 2], F32, tag="gmr", name="gmr")
                nc.vector.tensor_copy(gmr[:, 0:1], gm[:, 0:1])
                nc.vector.tensor_copy(gmr[:, 1:2], rstd)
                bps = p1ps.tile([128, 2], F32, tag="bps", name="bps", bufs=2)
                nc.tensor.matmul(bps, gt_sb, gmr, start=True, stop=True)
                bc = p1sb.tile([128, 2], F32, tag="bc", name="bc")
                nc.scalar.copy(bc, bps)
                nc.vector.tensor_tensor(out=scale_t[j], in0=bc[:, 1:2],
                                        in1=gamma_sb[:, j:j + 1],
                                        op=mybir.AluOpType.mult)
                mt = p1sb.tile([128, 1], F32, tag="mt", name="mt")
                nc.vector.tensor_tensor(out=mt, in0=bc[:, 0:1], in1=scale_t[j],
                                        op=mybir.AluOpType.mult)
                nc.vector.tensor_tensor(out=bias_t[j], in0=beta_sb[:, j:j + 1],
                                        in1=mt, op=mybir.AluOpType.subtract)

        if DEBUG:
            dsc = small.tile([128, 2], F32, tag="dsc", name="dsc")
            nc.vector.tensor_copy(dsc[:, 0:1], scale_t[0])
            nc.vector.tensor_copy(dsc[:, 1:2], bias_t[0])
            nc.sync.dma_start(out=dbg_sc[:, :], in_=dsc)

        # ================= P2: h_T windows -> K^T, V, Q^T =================
        def process_window(x_h, w, p2ps, p2sb, do_kv, qw=None):
            xt4 = []
            for i in range(4):
                xt = p2sb.tile([128, C], F32, tag="xtok2", name="xtok2", bufs=4)
                r0 = w * 512 + i * 128
                nc.sync.dma_start(out=xt, in_=x_h[r0:r0 + 128, :])
                xt4.append(xt)
            hw = []
            for j in range(NCH):
                psx = p2ps.tile([128, 512], F32, tag="tp2", name="tp2")
                for i in range(4):
                    nc.tensor.transpose(
                        psx[:, i * 128:(i + 1) * 128],
                        xt4[i][:, j * 128:(j + 1) * 128], ident)
                hwj = p2sb.tile([128, 512], F32R, tag=f"hw{j}", name=f"hw{j}")
                nc.vector.tensor_scalar(out=hwj, in0=psx, scalar1=scale_t[j],
                                        scalar2=bias_t[j],
                                        op0=mybir.AluOpType.mult,
                                        op1=mybir.AluOpType.add)
                hw.append(hwj)
            if DEBUG and do_kv and w == 0:
                nc.sync.dma_start(out=dbg_h[:, :], in_=hw[0].bitcast(F32))
            if do_kv:
                for ck in range(NCH):
                    ps = p2ps.tile([128, 512], F32, tag="kvp", name="kvp")
                    for ci in range(NCH):
                        nc.tensor.matmul(
                            ps, wk_t[ci][:, ck * 128:(ck + 1) * 128],
                            hw[ci], start=(ci == 0), stop=(ci == NCH - 1))
                    nc.any.tensor_copy(kt_t[ck][:, w * 512:(w + 1) * 512], ps)
                for i in range(4):
                    ps = p2ps.tile([128, 512], F32, tag="kvp", name="kvp")
                    for ci in range(NCH):
                        nc.tensor.matmul(
                            ps, hw[ci][:, i * 128:(i + 1) * 128],
                            wv_t[ci], start=(ci == 0), stop=(ci == NCH - 1))
                    nc.any.tensor_copy(v_big[:, w * 4 + i, :], ps)
            else:
                for cq in range(NCH):
                    ps = p2ps.tile([128, 512], F32, tag="kvp", name="kvp")
                    for ci in range(NCH):
                        nc.tensor.matmul(
                            ps, wq_t[ci][:, cq * 128:(cq + 1) * 128],
                            hw[ci], start=(ci == 0), stop=(ci == NCH - 1))
                    nc.scalar.activation(
                        qts_t[cq][:, qw * 512:(qw + 1) * 512], ps,
                        mybir.ActivationFunctionType.Identity,
                        bias=sbq[:, cq:cq + 1], scale=SCALE)

        with tc.tile_pool(name="p2ps", bufs=2, space="PSUM") as p2ps, \
             tc.tile_pool(name="p2sb", bufs=2) as p2sb:
            for w in range(NW):
                process_window(xkv_h, w, p2ps, p2sb, do_kv=True)
            for qw in range(NQW):
                process_window(xq_h, qw, p2ps, p2sb, do_kv=False, qw=qw)
            if DEBUG:
                nc.sync.dma_start(out=dbg_kt[:, :], in_=kt_t[0][:, 0:512].bitcast(F32))
                nc.sync.dma_start(out=dbg_q[:, :], in_=qts_t[0][:, 0:512].bitcast(F32))
        ctx2.close()

        # ================= P3: attention =================
        otspool = ctx.enter_context(tc.tile_pool(name="otspool", bufs=1))
        ots_t = [otspool.tile([128, NCH, 256], F32R, tag=f"ots{b}", name=f"ots{b}") for b in range(NBLK)]
        with tc.tile_pool(name="p3ps", bufs=2, space="PSUM") as p3ps, \
             tc.tile_pool(name="p3ot", bufs=2, space="PSUM") as p3ot, \
             tc.tile_pool(name="p3sb", bufs=6) as p3sb, \
             tc.tile_pool(name="p3ac", bufs=4) as p3ac:
            for blk in range(NBLK):
                q0 = blk * 256
                acc = [p3ac.tile([128, NW], F32, tag="acc", name="acc") for _ in range(2)]
                ot_ps = p3ot.tile([128, NCH, 256], F32, tag="ot", name="ot")
                pwin = [[None] * NW, [None] * NW]
                for w in range(NW):
                    for sub in range(2):
                        qs0 = q0 + sub * 128
                        ps_s = p3ps.tile([128, 512], F32, tag="sc", name="sc")
                        for cq in range(NCH):
                            nc.tensor.matmul(
                                ps_s, qts_t[cq][:, qs0:qs0 + 128],
                                kt_t[cq][:, w * 512:(w + 1) * 512],
                                start=(cq == 0), stop=(cq == NCH - 1))
                        pw = p3sb.tile([128, 512], F32, tag="pw", name="pw")
                        nc.scalar.activation(pw, ps_s,
                                             mybir.ActivationFunctionType.Exp,
                                             accum_out=acc[sub][:, w:w + 1])
                        pwin[sub][w] = pw
                        if DEBUG and blk == 0 and w == 0 and sub == 0:
                            nc.sync.dma_start(out=dbg_p[:, :], in_=pw)
                    for i in range(4):
                        w2 = w * 4 + i
                        pt_ps = p3ps.tile([128, 256], F32, tag="pt", name="pt")
                        for sub in range(2):
                            nc.tensor.transpose(
                                pt_ps[:, sub * 128:(sub + 1) * 128],
                                pwin[sub][w][:, i * 128:(i + 1) * 128], ident)
                        pts = p3sb.tile([128, 256], F32R, tag="pts", name="pts")
                        nc.any.tensor_copy(pts, pt_ps)
                        for cv in range(NCH):
                            # start only on the first matmul per PSUM bank:
                            # start=True zeroes has_written for the WHOLE bank,
                            # so the second region in each bank must not re-clear.
                            nc.tensor.matmul(
                                ot_ps[:, cv, :],
                                v_big[:, w2, cv * 128:(cv + 1) * 128],
                                pts, start=(w2 == 0 and cv % 2 == 0),
                                stop=(w2 == T // 128 - 1))
                for sub in range(2):
                    rs = p3ac.tile([128, 1], F32, tag="rs", name="rs")
                    nc.vector.reduce_sum(rs, acc[sub], axis=mybir.AxisListType.X)
                    nc.vector.reciprocal(rinv_t[blk * 2 + sub], rs)
                nc.any.tensor_copy(ots_t[blk], ot_ps)
                if DEBUG and blk == 0:
                    nc.sync.dma_start(out=dbg_ot[:, :, :], in_=ots_t[blk].bitcast(F32))
                    nc.sync.dma_start(out=dbg_r[:, :], in_=rinv_t[0])

        # ================= P4: projection + residual =================
        with tc.tile_pool(name="p4ps", bufs=2, space="PSUM") as p4ps, \
             tc.tile_pool(name="p4bp", bufs=1, space="PSUM") as p4bp, \
             tc.tile_pool(name="p4sb", bufs=3) as p4sb:
            bvp = p4bp.tile([1, C], F32, tag="bvp", name="bvp")
            for ci in range(NCH):
                nc.tensor.matmul(bvp, bv_sb[:, ci:ci + 1], wp_t[ci].bitcast(F32),
                                 start=(ci == 0), stop=(ci == NCH - 1))
            bpp = p4sb.tile([1, C], F32, tag="bpp", name="bpp")
            nc.vector.tensor_tensor(out=bpp, in0=bvp, in1=bp_row,
                                    op=mybir.AluOpType.add)
            bppb = p4sb.tile([128, C], F32, tag="bppb", name="bppb")
            nc.gpsimd.partition_broadcast(bppb, bpp[0:1, :])

            for blk in range(NBLK):
                for sub in range(2):
                    ti = blk * 2 + sub
                    ps_p = p4ps.tile([128, C], F32, tag="pj", name="pj")
                    for cv in range(NCH):
                        nc.tensor.matmul(
                            ps_p, ots_t[blk][:, cv, sub * 128:(sub + 1) * 128],
                            wp_t[cv], start=(cv == 0), stop=(cv == NCH - 1))
                    xres = p4sb.tile([128, C], F32, tag="xres", name="xres")
                    nc.sync.dma_start(out=xres, in_=xq_h[ti * 128:(ti + 1) * 128, :])
                    tmp = p4sb.tile([128, C], F32, tag="tmp", name="tmp")
                    nc.vector.scalar_tensor_tensor(
                        out=tmp, in0=ps_p, scalar=rinv_t[ti], in1=xres,
                        op0=mybir.AluOpType.mult, op1=mybir.AluOpType.add)
                    fin = p4sb.tile([128, C], F32, tag="fin", name="fin")
                    nc.vector.tensor_tensor(out=fin, in0=tmp, in1=bppb,
                                            op=mybir.AluOpType.add)
                    nc.sync.dma_start(out=out_h[ti * 128:(ti + 1) * 128, :], in_=fin)

    nc.compile()
    return nc


_NC_CACHE = []


def kernel(x, gamma, beta, wq, bq, wk, bk, wv, bv, wp, bp):
    x = np.ascontiguousarray(np.asarray(x, dtype=np.float32))
    if not _NC_CACHE:
        _NC_CACHE.append(_build())
    nc = _NC_CACHE[0]

    gm = np.zeros((128, 8), np.float32)
    for c in range(128):
        gm[c, c // GSIZE] = 1.0 / GSIZE
    gtm = np.zeros((8, 128), np.float32)
    for c in range(128):
        gtm[c // GSIZE, c] = 1.0
    common = {
        "wq": np.asarray(wq, np.float32), "wk": np.asarray(wk, np.float32),
        "wv": np.asarray(wv, np.float32), "wp": np.asarray(wp, np.float32),
        "bq": np.asarray(bq, np.float32), "bv": np.asarray(bv, np.float32),
        "bp": np.asarray(bp, np.float32),
        "gamma": np.asarray(gamma, np.float32),
        "beta": np.asarray(beta, np.float32),
        "ident": np.eye(128, dtype=np.float32),
        "gmat": gm, "gtmat": gtm,
    }
    xf = x.reshape(B, T, C)
    in_maps = []
    for core in range(NCORES):
        b, qoff = core // 4, (core % 4) * QS
        in_maps.append({
            **common,
            "xkv": xf[b],
            "xq": xf[b, qoff:qoff + QS],
        })
    res = run_bass_kernel_spmd(nc, in_maps, list(range(NCORES)))
    out = np.empty((B, T, C), np.float32)
    for core in range(NCORES):
        b, qoff = core // 4, (core % 4) * QS
        out[b, qoff:qoff + QS] = res.results[core]["out"]
    return out.reshape(B, H, W, C)


# revision 17
# speedup vs baseline: 1.0771x; 1.0771x over previous
"""AttentionBlock (GroupNorm + single-head full attention + residual) on 8 trn2 cores.

Sharding: core i -> batch i//4, query strip (i%4)*1024 .. +1024.
Each core computes its batch's full K/V (duplicated across the 4 cores sharing
the batch) so no inter-core communication is needed.

All matmuls run as float32r (full-rate fp32 variant); softmax in fp32.
Softmax skips max-subtraction (scores are O(+-10) with normalized inputs, and
softmax is shift-invariant so the result matches the reference), k-bias is
dropped (shift-invariant), v/proj biases are folded into a rank-1 post-
projection bias, and the softmax row normalization is deferred to the
projection output (row scaling commutes through out @ wp).
"""

import numpy as np
from contextlib import ExitStack

import concourse.bass as bass
import concourse.bacc as bacc
import concourse.tile as tile
from concourse import mybir
from concourse.bass_utils import run_bass_kernel_spmd

B, H, W, C = 2, 64, 64, 512
T = H * W                 # 4096 tokens per batch
NCORES = 8
QS = 1024                 # queries per core
GROUPS, GSIZE = 32, 16    # 8 groups per 128-channel chunk
EPS = 1e-5
SCALE = float(C) ** -0.5
F32 = mybir.dt.float32
F32R = mybir.dt.float32r
import os
if os.environ.get('KERNEL_MM_F32'):
    F32R = mybir.dt.float32
NCH = C // 128            # 4 channel chunks
NW = T // 512             # 8 token windows per batch
NQW = QS // 512           # 2 query windows per core
NBLK = QS // 512          # 2 attention q-blocks of 512 queries
NSUB = 4                  # 128-query subtiles per block


def _r(ap):
    return ap.bitcast(F32R)


def _build():
    nc = bacc.Bacc(None, target_bir_lowering=False)

    xkv_h = nc.declare_dram_parameter("xkv", [T, C], F32, isOutput=False)
    xq_h = nc.declare_dram_parameter("xq", [QS, C], F32, isOutput=False)
    wq_h = nc.declare_dram_parameter("wq", [C, C], F32R, isOutput=False)
    wk_h = nc.declare_dram_parameter("wk", [C, C], F32R, isOutput=False)
    wv_h = nc.declare_dram_parameter("wv", [C, C], F32R, isOutput=False)
    wp_h = nc.declare_dram_parameter("wp", [C, C], F32R, isOutput=False)
    bq_h = nc.declare_dram_parameter("bq", [C], F32, isOutput=False)
    bv_h = nc.declare_dram_parameter("bv", [C], F32, isOutput=False)
    bp_h = nc.declare_dram_parameter("bp", [C], F32, isOutput=False)
    gamma_h = nc.declare_dram_parameter("gamma", [C], F32, isOutput=False)
    beta_h = nc.declare_dram_parameter("beta", [C], F32, isOutput=False)
    ident_h = nc.declare_dram_parameter("ident", [128, 128], F32, isOutput=False)
    g_h = nc.declare_dram_parameter("gmat", [128, 8], F32, isOutput=False)
    gt_h = nc.declare_dram_parameter("gtmat", [8, 128], F32, isOutput=False)
    out_h = nc.declare_dram_parameter("out", [QS, C], F32, isOutput=True)
    DEBUG = bool(os.environ.get("KERNEL_DEBUG"))
    if DEBUG:
        dbg_h = nc.declare_dram_parameter("dbg_h", [128, 512], F32, isOutput=True)
        dbg_kt = nc.declare_dram_parameter("dbg_kt", [128, 512], F32, isOutput=True)
        dbg_q = nc.declare_dram_parameter("dbg_q", [128, 512], F32, isOutput=True)
        dbg_p = nc.declare_dram_parameter("dbg_p", [128, 512], F32, isOutput=True)
        dbg_ot = nc.declare_dram_parameter("dbg_ot", [128, 4, 256], F32, isOutput=True)
        dbg_r = nc.declare_dram_parameter("dbg_r", [128, 1], F32, isOutput=True)
        dbg_sc = nc.declare_dram_parameter("dbg_sc", [128, 2], F32, isOutput=True)

    with tile.TileContext(nc) as tc, ExitStack() as ctx:
        persist = ctx.enter_context(tc.tile_pool(name="persist", bufs=1))
        small = ctx.enter_context(tc.tile_pool(name="small", bufs=1))

        # ---- constants / weights ----
        ident = persist.tile([128, 128], F32, tag="ident", name="ident")
        nc.sync.dma_start(out=ident, in_=ident_h[:, :])
        identb = persist.tile([128, 128], mybir.dt.bfloat16, tag="identb", name="identb")
        nc.gpsimd.dma_start(out=identb, in_=ident_h[:, :])
        g_sb = persist.tile([128, 8], F32, tag="gmat", name="gmat")
        nc.sync.dma_start(out=g_sb, in_=g_h[:, :])
        gt_sb = persist.tile([8, 128], F32, tag="gtmat", name="gtmat")
        nc.sync.dma_start(out=gt_sb, in_=gt_h[:, :])

        bigpool = ctx.enter_context(tc.tile_pool(name="bigpool", bufs=1))
        kt_t = [bigpool.tile([128, T], F32R, tag=f"kt{j}", name=f"kt{j}") for j in range(NCH)]
        qts_t = [bigpool.tile([128, QS], F32R, tag=f"qts{j}", name=f"qts{j}") for j in range(NCH)]
        v_big = bigpool.tile([128, T // 128, C], F32R, tag="vbig", name="vbig")
        ctx2 = ExitStack()
        wpool = ctx2.enter_context(tc.tile_pool(name="wpool", bufs=1))
        wq_t = [wpool.tile([128, C], F32R, tag=f"wq{j}", name=f"wq{j}") for j in range(NCH)]
        wk_t = [wpool.tile([128, C], F32R, tag=f"wk{j}", name=f"wk{j}") for j in range(NCH)]
        wv_t = [wpool.tile([128, C], F32R, tag=f"wv{j}", name=f"wv{j}") for j in range(NCH)]
        wp_t = [persist.tile([128, C], F32R, tag=f"wp{j}", name=f"wp{j}") for j in range(NCH)]
        for j in range(NCH):
            sl = slice(j * 128, (j + 1) * 128)
            nc.scalar.dma_start(out=wq_t[j], in_=wq_h[sl, :])
            nc.scalar.dma_start(out=wk_t[j], in_=wk_h[sl, :])
            nc.scalar.dma_start(out=wv_t[j], in_=wv_h[sl, :])
            nc.scalar.dma_start(out=wp_t[j], in_=wp_h[sl, :])

        # per-channel vectors as [128, NCH] (column j = channel chunk j)
        def vec_tile(h, name):
            t = small.tile([128, NCH], F32, tag=name)
            nc.scalar.dma_start(out=t, in_=h.rearrange("(a p) -> p a", p=128))
            return t

        gamma_sb = vec_tile(gamma_h, "gamma")
        beta_sb = vec_tile(beta_h, "beta")
        bq_sb = vec_tile(bq_h, "bq")
        bv_sb = vec_tile(bv_h, "bv")
        bp_row = small.tile([1, C], F32, tag="bprow", name="bprow")
        nc.scalar.dma_start(out=bp_row, in_=bp_h.rearrange("(a c) -> a c", a=1))

        sbq = small.tile([128, NCH], F32, tag="sbq", name="sbq")
        nc.vector.tensor_scalar_mul(sbq, bq_sb, SCALE)
        eps8 = small.tile([8, 1], F32, tag="eps8", name="eps8")
        nc.vector.memset(eps8, EPS)

        rinv_t = [small.tile([128, 1], F32, tag=f"rinv{s}", name=f"rinv{s}") for s in range(NSUB * NBLK)]

        scale_t = [small.tile([128, 1], F32, tag=f"gnsc{j}", name=f"gnsc{j}") for j in range(NCH)]
        bias_t = [small.tile([128, 1], F32, tag=f"gnbi{j}", name=f"gnbi{j}") for j in range(NCH)]

        # ================= P1: group-norm statistics =================
        with tc.tile_pool(name="p1ps", bufs=1, space="PSUM") as p1ps, \
             tc.tile_pool(name="p1sb", bufs=2) as p1sb, \
             tc.tile_pool(name="statsb", bufs=1) as statsb:
            stats_t = [statsb.tile([128, NW, 6], F32, tag=f"st{j}", name=f"st{j}") for j in range(NCH)]
            for w in range(NW):
                xt4 = []
                for i in range(4):
                    xt = p1sb.tile([128, C], mybir.dt.bfloat16, tag="xtok",
                                   name="xtok", bufs=8)
                    r0 = w * 512 + i * 128
                    nc.gpsimd.dma_start(out=xt, in_=xkv_h[r0:r0 + 128, :])
                    xt4.append(xt)
                for j in range(NCH):
                    psx = p1ps.tile([128, 512], mybir.dt.bfloat16, tag="tp",
                                    name="tp", bufs=4)
                    for i in range(4):
                        nc.tensor.transpose(
                            psx[:, i * 128:(i + 1) * 128],
                            xt4[i][:, j * 128:(j + 1) * 128], identb)
                    nc.vector.bn_stats(out=stats_t[j][:, w, :], in_=psx)

            for j in range(NCH):
                mv = p1sb.tile([128, 2], F32, tag="mv", name="mv")
                nc.vector.bn_aggr(out=mv, in_=stats_t[j])
                msq = p1sb.tile([128, 1], F32, tag="msq", name="msq")
                nc.vector.tensor_tensor(out=msq, in0=mv[:, 0:1], in1=mv[:, 0:1],
                                        op=mybir.AluOpType.mult)
                smm = p1sb.tile([128, 2], F32, tag="smm", name="smm")
                nc.vector.tensor_copy(smm[:, 0:1], mv[:, 0:1])
                nc.vector.tensor_tensor(out=smm[:, 1:2], in0=mv[:, 1:2], in1=msq,
                                        op=mybir.AluOpType.add)
                gps = p1ps.tile([8, 2], F32, tag="grp", name="grp", bufs=2)
                nc.tensor.matmul(gps, g_sb, smm, start=True, stop=True)
                gm = p1sb.tile([8, 2], F32, tag="gm", name="gm")
                nc.scalar.copy(gm, gps)
                gmsq = p1sb.tile([8, 1], F32, tag="gmsq", name="gmsq")
                nc.vector.tensor_tensor(out=gmsq, in0=gm[:, 0:1], in1=gm[:, 0:1],
                                        op=mybir.AluOpType.mult)
                vg = p1sb.tile([8, 1], F32, tag="vg", name="vg")
                nc.vector.tensor_tensor(out=vg, in0=gm[:, 1:2], in1=gmsq,
                                        op=mybir.AluOpType.subtract)
                ve = p1sb.tile([8, 1], F32, tag="ve", name="ve")
                nc.vector.tensor_scalar_add(ve, vg, EPS)
                sd = p1sb.tile([8, 1], F32, tag="sd", name="sd")
                nc.scalar.activation(sd, ve, mybir.ActivationFunctionType.Sqrt)
                y0 = p1sb.tile([8, 1], F32, tag="y0", name="y0")
                nc.vector.reciprocal(y0, sd)
                # Newton-Raphson: y = y0 * (1.5 - 0.5 * v * y0^2)
                t1 = p1sb.tile([8, 1], F32, tag="t1", name="t1")
                nc.vector.tensor_tensor(out=t1, in0=ve, in1=y0,
                                        op=mybir.AluOpType.mult)
                nc.vector.tensor_tensor(out=t1, in0=t1, in1=y0,
                                        op=mybir.AluOpType.mult)
                nc.vector.tensor_scalar(out=t1, in0=t1, scalar1=-0.5, scalar2=1.5,
                                        op0=mybir.AluOpType.mult,
                                        op1=mybir.AluOpType.add)
                rstd = p1sb.tile([8, 1], F32, tag="rstd", name="rstd")
                nc.vector.tensor_tensor(out=rstd, in0=y0, in1=t1,
                                        op=mybir.AluOpType.mult)
                gmr = p1sb.tile([8, 2], F32, tag="gmr", name="gmr")
                nc.vector.tensor_copy(gmr[:, 0:1], gm[:, 0:1])
                nc.vector.tensor_copy(gmr[:, 1:2], rstd)
                bps = p1ps.tile([128, 2], F32, tag="bps", name="bps", bufs=2)
                nc.tensor.matmul(bps, gt_sb, gmr, start=True, stop=True)
                bc = p1sb.tile([128, 2], F32, tag="bc", name="bc")
                nc.scalar.copy(bc, bps)
                nc.vector.tensor_tensor(out=scale_t[j], in0=bc[:, 1:2],
                                        in1=gamma_sb[:, j:j + 1],
                                        op=mybir.AluOpType.mult)
                mt = p1sb.tile([128, 1], F32, tag="mt", name="mt")
                nc.vector.tensor_tensor(out=mt, in0=bc[:, 0:1], in1=scale_t[j],
                                        op=mybir.AluOpType.mult)
                nc.vector.tensor_tensor(out=bias_t[j], in0=beta_sb[:, j:j + 1],
                                        in1=mt, op=mybir.AluOpType.subtract)

        if DEBUG:
            dsc = small.tile([128, 2], F32, tag="dsc", name="dsc")
            nc.vector.tensor_copy(dsc[:, 0:1], scale_t[0])
            nc.vector.tensor_copy(dsc[:, 1:2], bias_t[0])
            nc.sync.dma_start(out=dbg_sc[:, :], in_=dsc)

        # ================= P2: h_T windows -> K^T, V, Q^T =================
        def process_window(x_h, w, p2ps, p2sb, do_kv, qw=None):
            xt4 = []
            for i in range(4):
                xt = p2sb.tile([128, C], F32, tag="xtok2", name="xtok2", bufs=4)
                r0 = w * 512 + i * 128
                nc.sync.dma_start(out=xt, in_=x_h[r0:r0 + 128, :])
                xt4.append(xt)
            hw = []
            for j in range(NCH):
                psx = p2ps.tile([128, 512], F32, tag="tp2", name="tp2")
                for i in range(4):
                    nc.tensor.transpose(
                        psx[:, i * 128:(i + 1) * 128],
                        xt4[i][:, j * 128:(j + 1) * 128], ident)
                hwj = p2sb.tile([128, 512], F32R, tag=f"hw{j}", name=f"hw{j}")
                nc.vector.tensor_scalar(out=hwj, in0=psx, scalar1=scale_t[j],
                                        scalar2=bias_t[j],
                                        op0=mybir.AluOpType.mult,
                                        op1=mybir.AluOpType.add)
                hw.append(hwj)
            if DEBUG and do_kv and w == 0:
                nc.sync.dma_start(out=dbg_h[:, :], in_=hw[0].bitcast(F32))
            if do_kv:
                for ck in range(NCH):
                    ps = p2ps.tile([128, 512], F32, tag="kvp", name="kvp")
                    for ci in range(NCH):
                        nc.tensor.matmul(
                            ps, wk_t[ci][:, ck * 128:(ck + 1) * 128],
                            hw[ci], start=(ci == 0), stop=(ci == NCH - 1))
                    nc.any.tensor_copy(kt_t[ck][:, w * 512:(w + 1) * 512], ps)
                for i in range(4):
                    ps = p2ps.tile([128, 512], F32, tag="kvp", name="kvp")
                    for ci in range(NCH):
                        nc.tensor.matmul(
                            ps, hw[ci][:, i * 128:(i + 1) * 128],
                            wv_t[ci], start=(ci == 0), stop=(ci == NCH - 1))
                    nc.any.tensor_copy(v_big[:, w * 4 + i, :], ps)
            else:
                for cq in range(NCH):
                    ps = p2ps.tile([128, 512], F32, tag="kvp", name="kvp")
                    for ci in range(NCH):
                        nc.tensor.matmul(
                            ps, wq_t[ci][:, cq * 128:(cq + 1) * 128],
                            hw[ci], start=(ci == 0), stop=(ci == NCH - 1))
                    nc.scalar.activation(
                        qts_t[cq][:, qw * 512:(qw + 1) * 512], ps,
                        mybir.ActivationFunctionType.Identity,
                        bias=sbq[:, cq:cq + 1], scale=SCALE)

        with tc.tile_pool(name="p2ps", bufs=2, space="PSUM") as p2ps, \
             tc.tile_pool(name="p2sb", bufs=2) as p2sb:
            for w in range(NW):
                process_window(xkv_h, w, p2ps, p2sb, do_kv=True)
            for qw in range(NQW):
                process_window(xq_h, qw, p2ps, p2sb, do_kv=False, qw=qw)
            if DEBUG:
                nc.sync.dma_start(out=dbg_kt[:, :], in_=kt_t[0][:, 0:512].bitcast(F32))
                nc.sync.dma_start(out=dbg_q[:, :], in_=qts_t[0][:, 0:512].bitcast(F32))
        ctx2.close()

        # ================= P3: attention =================
        otspool = ctx.enter_context(tc.tile_pool(name="otspool", bufs=1))
        ots_t = [otspool.tile([128, NCH, 512], F32R, tag=f"ots{b}", name=f"ots{b}") for b in range(NBLK)]
        with tc.tile_pool(name="p3ps", bufs=1, space="PSUM") as p3ps, \
             tc.tile_pool(name="p3ot", bufs=1, space="PSUM") as p3ot, \
             tc.tile_pool(name="p3sb", bufs=1) as p3sb, \
             tc.tile_pool(name="p3ac", bufs=4) as p3ac:
            for blk in range(NBLK):
                q0 = blk * 512
                acc = [p3ac.tile([128, NW], F32, tag="acc", name="acc") for _ in range(NSUB)]
                ot_ps = p3ot.tile([128, NCH, 512], F32, tag="ot", name="ot", bufs=1)
                pwin = [[None] * NW for _ in range(NSUB)]
                for w in range(NW):
                    for sub in range(NSUB):
                        qs0 = q0 + sub * 128
                        ps_s = p3ps.tile([128, 512], F32, tag="sc", name="sc", bufs=2)
                        for cq in range(NCH):
                            nc.tensor.matmul(
                                ps_s, qts_t[cq][:, qs0:qs0 + 128],
                                kt_t[cq][:, w * 512:(w + 1) * 512],
                                start=(cq == 0), stop=(cq == NCH - 1))
                        pw = p3sb.tile([128, 512], F32, tag="pw", name="pw", bufs=10)
                        nc.scalar.activation(pw, ps_s,
                                             mybir.ActivationFunctionType.Exp,
                                             accum_out=acc[sub][:, w:w + 1])
                        pwin[sub][w] = pw
                    for i in range(4):
                        w2 = w * 4 + i
                        pt_ps = p3ps.tile([128, 512], F32, tag="pt", name="pt", bufs=2)
                        for sub in range(NSUB):
                            nc.tensor.transpose(
                                pt_ps[:, sub * 128:(sub + 1) * 128],
                                pwin[sub][w][:, i * 128:(i + 1) * 128], ident)
                        pts = p3sb.tile([128, 512], F32R, tag="pts", name="pts", bufs=4)
                        nc.any.tensor_copy(pts, pt_ps)
                        for cv in range(NCH):
                            # each cv region is a full PSUM bank here, so every
                            # region starts its own accumulation group
                            nc.tensor.matmul(
                                ot_ps[:, cv, :],
                                v_big[:, w2, cv * 128:(cv + 1) * 128],
                                pts, start=(w2 == 0),
                                stop=(w2 == T // 128 - 1))
                for sub in range(NSUB):
                    rs = p3ac.tile([128, 1], F32, tag="rs", name="rs")
                    nc.vector.reduce_sum(rs, acc[sub], axis=mybir.AxisListType.X)
                    nc.vector.reciprocal(rinv_t[blk * NSUB + sub], rs)
                nc.any.tensor_copy(ots_t[blk], ot_ps)

        # ================= P4: projection + residual =================
        with tc.tile_pool(name="p4ps", bufs=2, space="PSUM") as p4ps, \
             tc.tile_pool(name="p4bp", bufs=1, space="PSUM") as p4bp, \
             tc.tile_pool(name="p4sb", bufs=3) as p4sb:
            bvp = p4bp.tile([1, C], F32, tag="bvp", name="bvp")
            for ci in range(NCH):
                nc.tensor.matmul(bvp, bv_sb[:, ci:ci + 1], wp_t[ci].bitcast(F32),
                                 start=(ci == 0), stop=(ci == NCH - 1))
            bpp = p4sb.tile([1, C], F32, tag="bpp", name="bpp")
            nc.vector.tensor_tensor(out=bpp, in0=bvp, in1=bp_row,
                                    op=mybir.AluOpType.add)
            bppb = p4sb.tile([128, C], F32, tag="bppb", name="bppb")
            nc.gpsimd.partition_broadcast(bppb, bpp[0:1, :])

            for blk in range(NBLK):
                for sub in range(NSUB):
                    ti = blk * NSUB + sub
                    ps_p = p4ps.tile([128, C], F32, tag="pj", name="pj")
                    for cv in range(NCH):
                        nc.tensor.matmul(
                            ps_p, ots_t[blk][:, cv, sub * 128:(sub + 1) * 128],
                            wp_t[cv], start=(cv == 0), stop=(cv == NCH - 1))
                    xres = p4sb.tile([128, C], F32, tag="xres", name="xres")
                    nc.sync.dma_start(out=xres, in_=xq_h[ti * 128:(ti + 1) * 128, :])
                    tmp = p4sb.tile([128, C], F32, tag="tmp", name="tmp")
                    nc.vector.scalar_tensor_tensor(
                        out=tmp, in0=ps_p, scalar=rinv_t[ti], in1=xres,
                        op0=mybir.AluOpType.mult, op1=mybir.AluOpType.add)
                    fin = p4sb.tile([128, C], F32, tag="fin", name="fin")
                    nc.vector.tensor_tensor(out=fin, in0=tmp, in1=bppb,
                                            op=mybir.AluOpType.add)
                    nc.sync.dma_start(out=out_h[ti * 128:(ti + 1) * 128, :], in_=fin)

    nc.compile()
    return nc


_NC_CACHE = []


def kernel(x, gamma, beta, wq, bq, wk, bk, wv, bv, wp, bp):
    x = np.ascontiguousarray(np.asarray(x, dtype=np.float32))
    if not _NC_CACHE:
        _NC_CACHE.append(_build())
    nc = _NC_CACHE[0]

    gm = np.zeros((128, 8), np.float32)
    for c in range(128):
        gm[c, c // GSIZE] = 1.0 / GSIZE
    gtm = np.zeros((8, 128), np.float32)
    for c in range(128):
        gtm[c // GSIZE, c] = 1.0
    common = {
        "wq": np.asarray(wq, np.float32), "wk": np.asarray(wk, np.float32),
        "wv": np.asarray(wv, np.float32), "wp": np.asarray(wp, np.float32),
        "bq": np.asarray(bq, np.float32), "bv": np.asarray(bv, np.float32),
        "bp": np.asarray(bp, np.float32),
        "gamma": np.asarray(gamma, np.float32),
        "beta": np.asarray(beta, np.float32),
        "ident": np.eye(128, dtype=np.float32),
        "gmat": gm, "gtmat": gtm,
    }
    xf = x.reshape(B, T, C)
    in_maps = []
    for core in range(NCORES):
        b, qoff = core // 4, (core % 4) * QS
        in_maps.append({
            **common,
            "xkv": xf[b],
            "xq": xf[b, qoff:qoff + QS],
        })
    res = run_bass_kernel_spmd(nc, in_maps, list(range(NCORES)))
    out = np.empty((B, T, C), np.float32)
    for core in range(NCORES):
        b, qoff = core // 4, (core % 4) * QS
        out[b, qoff:qoff + QS] = res.results[core]["out"]
    return out.reshape(B, H, W, C)


# revision 19
# speedup vs baseline: 1.4206x; 1.3189x over previous
"""AttentionBlock (GroupNorm + single-head full attention + residual) on 8 trn2 cores.

Sharding: core i -> batch i//4, query strip (i%4)*1024 .. +1024.
Each core computes its batch's full K/V (duplicated across the 4 cores sharing
the batch) so no inter-core communication is needed.

All matmuls run as float32r (full-rate fp32 variant); softmax in fp32.
Softmax skips max-subtraction (scores are O(+-10) with normalized inputs, and
softmax is shift-invariant so the result matches the reference), k-bias is
dropped (shift-invariant), v/proj biases are folded into a rank-1 post-
projection bias, and the softmax row normalization is deferred to the
projection output (row scaling commutes through out @ wp).
"""

import numpy as np
from contextlib import ExitStack

import concourse.bass as bass
import concourse.bacc as bacc
import concourse.tile as tile
from concourse import mybir
from concourse.bass_utils import run_bass_kernel_spmd

B, H, W, C = 2, 64, 64, 512
T = H * W                 # 4096 tokens per batch
NCORES = 8
QS = 1024                 # queries per core
GROUPS, GSIZE = 32, 16    # 8 groups per 128-channel chunk
EPS = 1e-5
SCALE = float(C) ** -0.5
F32 = mybir.dt.float32
F32R = mybir.dt.float32r
import os
if os.environ.get('KERNEL_MM_F32'):
    F32R = mybir.dt.float32
BF16 = mybir.dt.bfloat16
DT_ATT = F32R if os.environ.get('KERNEL_F32R') else BF16
NCH = C // 128            # 4 channel chunks
NW = T // 512             # 8 token windows per batch
NQW = QS // 512           # 2 query windows per core
NBLK = QS // 512          # 2 attention q-blocks of 512 queries
NSUB = 4                  # 128-query subtiles per block


def _r(ap):
    return ap.bitcast(F32R)


def _build():
    nc = bacc.Bacc(None, target_bir_lowering=False)

    xkv_h = nc.declare_dram_parameter("xkv", [T, C], F32, isOutput=False)
    xq_h = nc.declare_dram_parameter("xq", [QS, C], F32, isOutput=False)
    wq_h = nc.declare_dram_parameter("wq", [C, C], DT_ATT, isOutput=False)
    wk_h = nc.declare_dram_parameter("wk", [C, C], DT_ATT, isOutput=False)
    wv_h = nc.declare_dram_parameter("wv", [C, C], DT_ATT, isOutput=False)
    wp_h = nc.declare_dram_parameter("wp", [C, C], F32R, isOutput=False)
    bq_h = nc.declare_dram_parameter("bq", [C], F32, isOutput=False)
    bv_h = nc.declare_dram_parameter("bv", [C], F32, isOutput=False)
    bp_h = nc.declare_dram_parameter("bp", [C], F32, isOutput=False)
    gamma_h = nc.declare_dram_parameter("gamma", [C], F32, isOutput=False)
    beta_h = nc.declare_dram_parameter("beta", [C], F32, isOutput=False)
    ident_h = nc.declare_dram_parameter("ident", [128, 128], F32, isOutput=False)
    g_h = nc.declare_dram_parameter("gmat", [128, 8], F32, isOutput=False)
    gt_h = nc.declare_dram_parameter("gtmat", [8, 128], F32, isOutput=False)
    out_h = nc.declare_dram_parameter("out", [QS, C], F32, isOutput=True)
    DEBUG = bool(os.environ.get("KERNEL_DEBUG"))
    if DEBUG:
        dbg_h = nc.declare_dram_parameter("dbg_h", [128, 512], F32, isOutput=True)
        dbg_kt = nc.declare_dram_parameter("dbg_kt", [128, 512], F32, isOutput=True)
        dbg_q = nc.declare_dram_parameter("dbg_q", [128, 512], F32, isOutput=True)
        dbg_p = nc.declare_dram_parameter("dbg_p", [128, 512], F32, isOutput=True)
        dbg_ot = nc.declare_dram_parameter("dbg_ot", [128, 4, 256], F32, isOutput=True)
        dbg_r = nc.declare_dram_parameter("dbg_r", [128, 1], F32, isOutput=True)
        dbg_sc = nc.declare_dram_parameter("dbg_sc", [128, 2], F32, isOutput=True)

    with tile.TileContext(nc) as tc, ExitStack() as ctx:
        persist = ctx.enter_context(tc.tile_pool(name="persist", bufs=1))
        small = ctx.enter_context(tc.tile_pool(name="small", bufs=1))

        # ---- constants / weights ----
        ident = persist.tile([128, 128], F32, tag="ident", name="ident")
        nc.sync.dma_start(out=ident, in_=ident_h[:, :])
        identb = persist.tile([128, 128], mybir.dt.bfloat16, tag="identb", name="identb")
        nc.gpsimd.dma_start(out=identb, in_=ident_h[:, :])
        g_sb = persist.tile([128, 8], F32, tag="gmat", name="gmat")
        nc.sync.dma_start(out=g_sb, in_=g_h[:, :])
        gt_sb = persist.tile([8, 128], F32, tag="gtmat", name="gtmat")
        nc.sync.dma_start(out=gt_sb, in_=gt_h[:, :])

        bigpool = ctx.enter_context(tc.tile_pool(name="bigpool", bufs=1))
        kt_t = [bigpool.tile([128, T], DT_ATT, tag=f"kt{j}", name=f"kt{j}") for j in range(NCH)]
        qts_t = [bigpool.tile([128, QS], DT_ATT, tag=f"qts{j}", name=f"qts{j}") for j in range(NCH)]
        v_big = bigpool.tile([128, T // 128, C], DT_ATT, tag="vbig", name="vbig")
        ctx2 = ExitStack()
        wpool = ctx2.enter_context(tc.tile_pool(name="wpool", bufs=1))
        wq_t = [wpool.tile([128, C], DT_ATT, tag=f"wq{j}", name=f"wq{j}") for j in range(NCH)]
        wk_t = [wpool.tile([128, C], DT_ATT, tag=f"wk{j}", name=f"wk{j}") for j in range(NCH)]
        wv_t = [wpool.tile([128, C], DT_ATT, tag=f"wv{j}", name=f"wv{j}") for j in range(NCH)]
        wp_t = [persist.tile([128, C], F32R, tag=f"wp{j}", name=f"wp{j}") for j in range(NCH)]
        for j in range(NCH):
            sl = slice(j * 128, (j + 1) * 128)
            nc.scalar.dma_start(out=wq_t[j], in_=wq_h[sl, :])
            nc.scalar.dma_start(out=wk_t[j], in_=wk_h[sl, :])
            nc.scalar.dma_start(out=wv_t[j], in_=wv_h[sl, :])
            nc.scalar.dma_start(out=wp_t[j], in_=wp_h[sl, :])

        # per-channel vectors as [128, NCH] (column j = channel chunk j)
        def vec_tile(h, name):
            t = small.tile([128, NCH], F32, tag=name)
            nc.scalar.dma_start(out=t, in_=h.rearrange("(a p) -> p a", p=128))
            return t

        gamma_sb = vec_tile(gamma_h, "gamma")
        beta_sb = vec_tile(beta_h, "beta")
        bq_sb = vec_tile(bq_h, "bq")
        bv_sb = vec_tile(bv_h, "bv")
        bp_row = small.tile([1, C], F32, tag="bprow", name="bprow")
        nc.scalar.dma_start(out=bp_row, in_=bp_h.rearrange("(a c) -> a c", a=1))

        sbq = small.tile([128, NCH], F32, tag="sbq", name="sbq")
        nc.vector.tensor_scalar_mul(sbq, bq_sb, SCALE)
        eps8 = small.tile([8, 1], F32, tag="eps8", name="eps8")
        nc.vector.memset(eps8, EPS)

        rinv_t = [small.tile([128, 1], F32, tag=f"rinv{s}", name=f"rinv{s}") for s in range(NSUB * NBLK)]

        scale_t = [small.tile([128, 1], F32, tag=f"gnsc{j}", name=f"gnsc{j}") for j in range(NCH)]
        bias_t = [small.tile([128, 1], F32, tag=f"gnbi{j}", name=f"gnbi{j}") for j in range(NCH)]

        # ================= P1: group-norm statistics =================
        with tc.tile_pool(name="p1ps", bufs=1, space="PSUM") as p1ps, \
             tc.tile_pool(name="p1sb", bufs=2) as p1sb, \
             tc.tile_pool(name="statsb", bufs=1) as statsb:
            stats_t = [statsb.tile([128, NW, 6], F32, tag=f"st{j}", name=f"st{j}") for j in range(NCH)]
            for w in range(NW):
                xt4 = []
                for i in range(4):
                    xt = p1sb.tile([128, C], mybir.dt.bfloat16, tag="xtok",
                                   name="xtok", bufs=8)
                    r0 = w * 512 + i * 128
                    nc.gpsimd.dma_start(out=xt, in_=xkv_h[r0:r0 + 128, :])
                    xt4.append(xt)
                for j in range(NCH):
                    psx = p1ps.tile([128, 512], mybir.dt.bfloat16, tag="tp",
                                    name="tp", bufs=4)
                    for i in range(4):
                        nc.tensor.transpose(
                            psx[:, i * 128:(i + 1) * 128],
                            xt4[i][:, j * 128:(j + 1) * 128], identb)
                    nc.vector.bn_stats(out=stats_t[j][:, w, :], in_=psx)

            for j in range(NCH):
                mv = p1sb.tile([128, 2], F32, tag="mv", name="mv")
                nc.vector.bn_aggr(out=mv, in_=stats_t[j])
                msq = p1sb.tile([128, 1], F32, tag="msq", name="msq")
                nc.vector.tensor_tensor(out=msq, in0=mv[:, 0:1], in1=mv[:, 0:1],
                                        op=mybir.AluOpType.mult)
                smm = p1sb.tile([128, 2], F32, tag="smm", name="smm")
                nc.vector.tensor_copy(smm[:, 0:1], mv[:, 0:1])
                nc.vector.tensor_tensor(out=smm[:, 1:2], in0=mv[:, 1:2], in1=msq,
                                        op=mybir.AluOpType.add)
                gps = p1ps.tile([8, 2], F32, tag="grp", name="grp", bufs=2)
                nc.tensor.matmul(gps, g_sb, smm, start=True, stop=True)
                gm = p1sb.tile([8, 2], F32, tag="gm", name="gm")
                nc.scalar.copy(gm, gps)
                gmsq = p1sb.tile([8, 1], F32, tag="gmsq", name="gmsq")
                nc.vector.tensor_tensor(out=gmsq, in0=gm[:, 0:1], in1=gm[:, 0:1],
                                        op=mybir.AluOpType.mult)
                vg = p1sb.tile([8, 1], F32, tag="vg", name="vg")
                nc.vector.tensor_tensor(out=vg, in0=gm[:, 1:2], in1=gmsq,
                                        op=mybir.AluOpType.subtract)
                ve = p1sb.tile([8, 1], F32, tag="ve", name="ve")
                nc.vector.tensor_scalar_add(ve, vg, EPS)
                sd = p1sb.tile([8, 1], F32, tag="sd", name="sd")
                nc.scalar.activation(sd, ve, mybir.ActivationFunctionType.Sqrt)
                y0 = p1sb.tile([8, 1], F32, tag="y0", name="y0")
                nc.vector.reciprocal(y0, sd)
                # Newton-Raphson: y = y0 * (1.5 - 0.5 * v * y0^2)
                t1 = p1sb.tile([8, 1], F32, tag="t1", name="t1")
                nc.vector.tensor_tensor(out=t1, in0=ve, in1=y0,
                                        op=mybir.AluOpType.mult)
                nc.vector.tensor_tensor(out=t1, in0=t1, in1=y0,
                                        op=mybir.AluOpType.mult)
                nc.vector.tensor_scalar(out=t1, in0=t1, scalar1=-0.5, scalar2=1.5,
                                        op0=mybir.AluOpType.mult,
                                        op1=mybir.AluOpType.add)
                rstd = p1sb.tile([8, 1], F32, tag="rstd", name="rstd")
                nc.vector.tensor_tensor(out=rstd, in0=y0, in1=t1,
                                        op=mybir.AluOpType.mult)
                gmr = p1sb.tile([8, 2], F32, tag="gmr", name="gmr")
                nc.vector.tensor_copy(gmr[:, 0:1], gm[:, 0:1])
                nc.vector.tensor_copy(gmr[:, 1:2], rstd)
                bps = p1ps.tile([128, 2], F32, tag="bps", name="bps", bufs=2)
                nc.tensor.matmul(bps, gt_sb, gmr, start=True, stop=True)
                bc = p1sb.tile([128, 2], F32, tag="bc", name="bc")
                nc.scalar.copy(bc, bps)
                nc.vector.tensor_tensor(out=scale_t[j], in0=bc[:, 1:2],
                                        in1=gamma_sb[:, j:j + 1],
                                        op=mybir.AluOpType.mult)
                mt = p1sb.tile([128, 1], F32, tag="mt", name="mt")
                nc.vector.tensor_tensor(out=mt, in0=bc[:, 0:1], in1=scale_t[j],
                                        op=mybir.AluOpType.mult)
                nc.vector.tensor_tensor(out=bias_t[j], in0=beta_sb[:, j:j + 1],
                                        in1=mt, op=mybir.AluOpType.subtract)

        if DEBUG:
            dsc = small.tile([128, 2], F32, tag="dsc", name="dsc")
            nc.vector.tensor_copy(dsc[:, 0:1], scale_t[0])
            nc.vector.tensor_copy(dsc[:, 1:2], bias_t[0])
            nc.sync.dma_start(out=dbg_sc[:, :], in_=dsc)

        # ================= P2: h_T windows -> K^T, V, Q^T =================
        def process_window(x_h, w, p2ps, p2sb, do_kv, qw=None):
            dt_x = BF16 if DT_ATT == BF16 else F32
            xt4 = []
            for i in range(4):
                xt = p2sb.tile([128, C], dt_x, tag="xtok2", name="xtok2", bufs=4)
                r0 = w * 512 + i * 128
                if DT_ATT == BF16:
                    nc.gpsimd.dma_start(out=xt, in_=x_h[r0:r0 + 128, :])
                else:
                    nc.sync.dma_start(out=xt, in_=x_h[r0:r0 + 128, :])
                xt4.append(xt)
            ident_att = identb if DT_ATT == BF16 else ident
            hw = []
            for j in range(NCH):
                psx = p2ps.tile([128, 512], dt_x, tag="tp2", name="tp2")
                for i in range(4):
                    nc.tensor.transpose(
                        psx[:, i * 128:(i + 1) * 128],
                        xt4[i][:, j * 128:(j + 1) * 128], ident_att)
                hwj = p2sb.tile([128, 512], DT_ATT, tag=f"hw{j}", name=f"hw{j}")
                nc.vector.tensor_scalar(out=hwj, in0=psx, scalar1=scale_t[j],
                                        scalar2=bias_t[j],
                                        op0=mybir.AluOpType.mult,
                                        op1=mybir.AluOpType.add)
                hw.append(hwj)
            if DEBUG and do_kv and w == 0:
                nc.sync.dma_start(out=dbg_h[:, :], in_=hw[0].bitcast(F32))
            if do_kv:
                for ck in range(NCH):
                    ps = p2ps.tile([128, 512], F32, tag="kvp", name="kvp")
                    for ci in range(NCH):
                        nc.tensor.matmul(
                            ps, wk_t[ci][:, ck * 128:(ck + 1) * 128],
                            hw[ci], start=(ci == 0), stop=(ci == NCH - 1))
                    nc.any.tensor_copy(kt_t[ck][:, w * 512:(w + 1) * 512], ps)
                for i in range(4):
                    ps = p2ps.tile([128, 512], F32, tag="kvp", name="kvp")
                    for ci in range(NCH):
                        nc.tensor.matmul(
                            ps, hw[ci][:, i * 128:(i + 1) * 128],
                            wv_t[ci], start=(ci == 0), stop=(ci == NCH - 1))
                    nc.any.tensor_copy(v_big[:, w * 4 + i, :], ps)
            else:
                for cq in range(NCH):
                    ps = p2ps.tile([128, 512], F32, tag="kvp", name="kvp")
                    for ci in range(NCH):
                        nc.tensor.matmul(
                            ps, wq_t[ci][:, cq * 128:(cq + 1) * 128],
                            hw[ci], start=(ci == 0), stop=(ci == NCH - 1))
                    nc.scalar.activation(
                        qts_t[cq][:, qw * 512:(qw + 1) * 512], ps,
                        mybir.ActivationFunctionType.Identity,
                        bias=sbq[:, cq:cq + 1], scale=SCALE)

        with tc.tile_pool(name="p2ps", bufs=2, space="PSUM") as p2ps, \
             tc.tile_pool(name="p2sb", bufs=2) as p2sb:
            for w in range(NW):
                process_window(xkv_h, w, p2ps, p2sb, do_kv=True)
            for qw in range(NQW):
                process_window(xq_h, qw, p2ps, p2sb, do_kv=False, qw=qw)
            if DEBUG:
                nc.sync.dma_start(out=dbg_kt[:, :], in_=kt_t[0][:, 0:512].bitcast(F32))
                nc.sync.dma_start(out=dbg_q[:, :], in_=qts_t[0][:, 0:512].bitcast(F32))
        ctx2.close()

        # ================= P3: attention =================
        otspool = ctx.enter_context(tc.tile_pool(name="otspool", bufs=1))
        ots_t = [otspool.tile([128, NCH, 512], F32R, tag=f"ots{b}", name=f"ots{b}") for b in range(NBLK)]
        with tc.tile_pool(name="p3ps", bufs=1, space="PSUM") as p3ps, \
             tc.tile_pool(name="p3ot", bufs=1, space="PSUM") as p3ot, \
             tc.tile_pool(name="p3sb", bufs=1) as p3sb, \
             tc.tile_pool(name="p3ac", bufs=4) as p3ac:
            for blk in range(NBLK):
                q0 = blk * 512
                acc = [p3ac.tile([128, NW], F32, tag="acc", name="acc") for _ in range(NSUB)]
                ot_ps = p3ot.tile([128, NCH, 512], F32, tag="ot", name="ot", bufs=1)
                pwin = [[None] * NW for _ in range(NSUB)]
                for w in range(NW):
                    for sub in range(NSUB):
                        qs0 = q0 + sub * 128
                        ps_s = p3ps.tile([128, 512], F32, tag="sc", name="sc", bufs=2)
                        for cq in range(NCH):
                            nc.tensor.matmul(
                                ps_s, qts_t[cq][:, qs0:qs0 + 128],
                                kt_t[cq][:, w * 512:(w + 1) * 512],
                                start=(cq == 0), stop=(cq == NCH - 1))
                        pw = p3sb.tile([128, 512], BF16 if DT_ATT == BF16 else F32, tag="pw", name="pw", bufs=10)
                        nc.scalar.activation(pw, ps_s,
                                             mybir.ActivationFunctionType.Exp,
                                             accum_out=acc[sub][:, w:w + 1])
                        pwin[sub][w] = pw
                    for i in range(4):
                        w2 = w * 4 + i
                        dt_p = BF16 if DT_ATT == BF16 else F32
                        pt_ps = p3ps.tile([128, 512], dt_p, tag="pt", name="pt", bufs=2)
                        ident_att = identb if DT_ATT == BF16 else ident
                        for sub in range(NSUB):
                            nc.tensor.transpose(
                                pt_ps[:, sub * 128:(sub + 1) * 128],
                                pwin[sub][w][:, i * 128:(i + 1) * 128], ident_att)
                        pts = p3sb.tile([128, 512], DT_ATT, tag="pts", name="pts", bufs=4)
                        nc.any.tensor_copy(pts, pt_ps)
                        for cv in range(NCH):
                            # each cv region is a full PSUM bank here, so every
                            # region starts its own accumulation group
                            nc.tensor.matmul(
                                ot_ps[:, cv, :],
                                v_big[:, w2, cv * 128:(cv + 1) * 128],
                                pts, start=(w2 == 0),
                                stop=(w2 == T // 128 - 1))
                for sub in range(NSUB):
                    rs = p3ac.tile([128, 1], F32, tag="rs", name="rs")
                    nc.vector.reduce_sum(rs, acc[sub], axis=mybir.AxisListType.X)
                    nc.vector.reciprocal(rinv_t[blk * NSUB + sub], rs)
                nc.any.tensor_copy(ots_t[blk], ot_ps)

        # ================= P4: projection + residual =================
        with tc.tile_pool(name="p4ps", bufs=2, space="PSUM") as p4ps, \
             tc.tile_pool(name="p4bp", bufs=1, space="PSUM") as p4bp, \
             tc.tile_pool(name="p4sb", bufs=3) as p4sb:
            bvp = p4bp.tile([1, C], F32, tag="bvp", name="bvp")
            for ci in range(NCH):
                nc.tensor.matmul(bvp, bv_sb[:, ci:ci + 1], wp_t[ci].bitcast(F32),
                                 start=(ci == 0), stop=(ci == NCH - 1))
            bpp = p4sb.tile([1, C], F32, tag="bpp", name="bpp")
            nc.vector.tensor_tensor(out=bpp, in0=bvp, in1=bp_row,
                                    op=mybir.AluOpType.add)
            bppb = p4sb.tile([128, C], F32, tag="bppb", name="bppb")
            nc.gpsimd.partition_broadcast(bppb, bpp[0:1, :])

            for blk in range(NBLK):
                for sub in range(NSUB):
                    ti = blk * NSUB + sub
                    ps_p = p4ps.tile([128, C], F32, tag="pj", name="pj")
                    for cv in range(NCH):
                        nc.tensor.matmul(
                            ps_p, ots_t[blk][:, cv, sub * 128:(sub + 1) * 128],
                            wp_t[cv], start=(cv == 0), stop=(cv == NCH - 1))
                    xres = p4sb.tile([128, C], F32, tag="xres", name="xres")
                    nc.sync.dma_start(out=xres, in_=xq_h[ti * 128:(ti + 1) * 128, :])
                    tmp = p4sb.tile([128, C], F32, tag="tmp", name="tmp")
                    nc.vector.scalar_tensor_tensor(
                        out=tmp, in0=ps_p, scalar=rinv_t[ti], in1=xres,
                        op0=mybir.AluOpType.mult, op1=mybir.AluOpType.add)
                    fin = p4sb.tile([128, C], F32, tag="fin", name="fin")
                    nc.vector.tensor_tensor(out=fin, in0=tmp, in1=bppb,
                                            op=mybir.AluOpType.add)
                    nc.sync.dma_start(out=out_h[ti * 128:(ti + 1) * 128, :], in_=fin)

    nc.compile()
    return nc


_NC_CACHE = []


def kernel(x, gamma, beta, wq, bq, wk, bk, wv, bv, wp, bp):
    x = np.ascontiguousarray(np.asarray(x, dtype=np.float32))
    if not _NC_CACHE:
        _NC_CACHE.append(_build())
    nc = _NC_CACHE[0]

    gm = np.zeros((128, 8), np.float32)
    for c in range(128):
        gm[c, c // GSIZE] = 1.0 / GSIZE
    gtm = np.zeros((8, 128), np.float32)
    for c in range(128):
        gtm[c // GSIZE, c] = 1.0
    import ml_dtypes
    wdt = np.float32 if os.environ.get('KERNEL_F32R') else ml_dtypes.bfloat16
    common = {
        "wq": np.asarray(wq, wdt), "wk": np.asarray(wk, wdt),
        "wv": np.asarray(wv, wdt), "wp": np.asarray(wp, np.float32),
        "bq": np.asarray(bq, np.float32), "bv": np.asarray(bv, np.float32),
        "bp": np.asarray(bp, np.float32),
        "gamma": np.asarray(gamma, np.float32),
        "beta": np.asarray(beta, np.float32),
        "ident": np.eye(128, dtype=np.float32),
        "gmat": gm, "gtmat": gtm,
    }
    xf = x.reshape(B, T, C)
    in_maps = []
    for core in range(NCORES):
        b, qoff = core // 4, (core % 4) * QS
        in_maps.append({
            **common,
            "xkv": xf[b],
            "xq": xf[b, qoff:qoff + QS],
        })
    res = run_bass_kernel_spmd(nc, in_maps, list(range(NCORES)))
    out = np.empty((B, T, C), np.float32)
    for core in range(NCORES):
        b, qoff = core // 4, (core % 4) * QS
        out[b, qoff:qoff + QS] = res.results[core]["out"]
    return out.reshape(B, H, W, C)
